# revision 51
# baseline (speedup 1.0000x reference)
"""Trainium2 Bass kernel for causal multi-head attention + output projection.

Problem (hardcoded): x[4, 2048, 1024] fp32, 16 heads, head_dim 64, causal,
torch-Linear convention (y = x @ W.T), output projection with bias.

Sharding over 8 NeuronCores: batch (4) x head-group (2 groups of 8 heads).
Each core computes q/k/v for its 8 heads of its batch, causal attention in
the S^T layout (keys on partitions, queries on free dim; softmax denominators
produced by an appended ones-column in V), then a partial output projection
over its 512 head-dims for all 2048 queries. The host sums the two partial
projections per batch and adds the bias (tensor-parallel unshard).

All matmul operands are bf16 (fp32 PSUM accumulation). S matmuls zero-pad the
64-dim head contraction to K=128 so every matmul runs the full 128x128 array
configuration (no PE tile-mode switches, which drain the array). Emission
round-robins independent GEMM work into the attention windows (k/q/v of the
second token half into query-half-0 attention, the first 8 projection tiles
into query-half-1 attention) so the in-order PE stream never starves while
the scalar engine works through the exp strips.
"""
import os
import sys
import types

import numpy as np

import concourse.bass as bass
import concourse.mybir as mybir
import concourse.tile as tile
from concourse import bacc, bass_utils

BF16 = mybir.dt.bfloat16
F32 = mybir.dt.float32
AF = mybir.ActivationFunctionType
OP = mybir.AluOpType

B, T, D = 4, 2048, 1024
H, HD = 16, 64
HG = 8          # heads per core
QH = T // 2     # query half
N_CORES = 8
SCALE = 1.0 / 8.0
SKIP_NORM = False


# ---------------------------------------------------------------------------
# environment glue
# ---------------------------------------------------------------------------

def _install_ntff_hook():
    if 'antenv.axon_hooks' in sys.modules:
        return
    try:
        from trn_agent_boot.trn_boot import _ntff_profile_via_ctypes
        hook = _ntff_profile_via_ctypes('/opt/axon/libaxon_pjrt.so')
    except Exception:
        hook = None
    mod = types.ModuleType('antenv.axon_hooks')
    mod.get_axon_ntff_profile_hook = lambda: hook
    mod.set_axon_ntff_profile_hook = lambda h: None
    sys.modules['antenv.axon_hooks'] = mod


def _run_spmd(nc, in_maps, trace=False):
    from concourse.bass_interp import get_hw_module
    bass_utils.upload_artifacts = lambda tmpdir: tmpdir
    if trace:
        _install_ntff_hook()
    old_m = nc.m
    nc.m = get_hw_module(nc.m)
    try:
        return bass_utils.run_bass_kernel_spmd(
            nc, in_maps, core_ids=list(range(N_CORES)),
            trace=trace, trace_cores=[0] if trace else None,
        )
    finally:
        nc.m = old_m


def _interleave(main_units, filler_units):
    """Emit main_units in order, spreading filler_units evenly between them."""
    n_main, n_fill = len(main_units), len(filler_units)
    fi = 0
    for i, u in enumerate(main_units):
        u()
        want = ((i + 1) * n_fill) // max(1, n_main)
        while fi < want:
            filler_units[fi]()
            fi += 1
    while fi < n_fill:
        filler_units[fi]()
        fi += 1


# ---------------------------------------------------------------------------
# kernel program
# ---------------------------------------------------------------------------

def _kqv_units(nc, hf, xh, wk_sb, wq_sb, wv_sb, qT_sb, kT_sb, v_sb,
               ps, evict_eng):
    """Unit closures for k, q projections of token-half hf and v of its 8
    token blocks. Each unit is ~4096 PE cycles into one psum bank."""
    cbase = hf * QH
    units = []
    if evict_eng is nc.scalar:
        evict = nc.scalar.copy
    else:
        evict = nc.vector.tensor_copy

    def kq_unit(w_sb, dst, padded, m, ch):
        pt = ps.tile([128, QH], F32, tag="s", name="pt")
        half = pt[:, 0:512]
        for kk in range(8):
            nc.tensor.matmul(
                half[:], lhsT=w_sb[:, kk, m * 128:(m + 1) * 128],
                rhs=xh[ch][:, kk, :], start=(kk == 0), stop=(kk == 7))
        cs = slice(cbase + ch * 512, cbase + (ch + 1) * 512)
        if padded:
            evict(dst[0:64, 2 * m, cs], half[0:64, :])
            evict(dst[64:128, 2 * m + 1, cs], half[64:128, :])
        else:
            evict(dst[:, m, cs], half[:])

    def v_unit(mm):
        m = hf * 8 + mm
        pt = ps.tile([128, QH], F32, tag="s", name="pt")
        half = pt[:, 0:512]
        for kk in range(8):
            nc.tensor.matmul(
                half[:],
                lhsT=xh[mm // 4][:, kk, (mm % 4) * 128:(mm % 4 + 1) * 128],
                rhs=wv_sb[:, kk, :], start=(kk == 0), stop=(kk == 7))
        evict(v_sb[:, m, :, 0:64],
              half[:].rearrange("p (h d) -> p h d", h=HG))

    for w_sb, dst, padded in ((wk_sb, kT_sb, True), (wq_sb, qT_sb, False)):
        for m in range(4):
            for ch in range(2):
                units.append(lambda w=w_sb, d=dst, p=padded, m=m, c=ch:
                             kq_unit(w, d, p, m, c))
    for mm in range(8):
        units.append(lambda mm=mm: v_unit(mm))
    return units


def _attn_units(nc, p, qh, qT_sb, kT_sb, v_sb, mask_sb, o_all,
                ps, es, zs, zdr, state, ones_sb=None):
    """Unit closures for heads (2p, 2p+1), query half qh: one unit per
    j-step (S + exp + mask + AV for both heads) plus an eviction unit."""
    heads = (2 * p, 2 * p + 1)
    jmax = 8 * qh + 8
    fast = ones_sb is not None  # last pair: broadcast 1/Z on the idle PE

    def evict_half(o_ps, b):
        # Columns [512b, 512b+512) of the o accumulators got their last AV
        # write at j = 8*qh + 3 + 4*b: evict un-normalized (frees the PSUM
        # bank early), then normalize in SBUF once the reciprocal
        # denominators come back from the DRAM broadcast bounce.
        cs = slice(QH * qh + 512 * b, QH * qh + 512 * (b + 1))
        ps_cs = slice(512 * b, 512 * (b + 1))
        zst = zs.tile([1, 2 * 512], F32, tag="zst", name="zst")
        # bank-1 eviction lands at the pair boundary where the scalar engine
        # sits between exp streams: run its copies there so ACT and DVE
        # drain the accumulators in parallel
        ocopy = nc.scalar.copy if b == 1 else nc.vector.tensor_copy
        for h in heads:
            pb = 64 * (h % 2)
            ocopy(o_all[pb:pb + 64, p, cs], o_ps[h][0:64, ps_cs])
            if not SKIP_NORM:
                nc.vector.tensor_copy(zst[:, (h % 2) * 512:(h % 2 + 1) * 512],
                                      o_ps[h][64:65, ps_cs])
        if SKIP_NORM:
            return
        zrc = zs.tile([1, 2 * 512], F32, tag="zrc", name="zrc")
        nc.vector.reciprocal_approx_fast(zrc[:], zst[:])
        if fast:
            zb_ps = ps.tile([128, QH], F32, tag="s", name="zbps")
            for c in (0, 512):
                nc.tensor.matmul(zb_ps[:, c:c + 512], lhsT=ones_sb[:],
                                 rhs=zrc[:, c:c + 512], start=True, stop=True)
            for h in heads:
                pb = 64 * (h % 2)
                zsl = slice((h % 2) * 512, (h % 2 + 1) * 512)
                nc.vector.tensor_tensor(o_all[pb:pb + 64, p, cs],
                                        o_all[pb:pb + 64, p, cs],
                                        zb_ps[pb:pb + 64, zsl], OP.mult)
            return
        rows = zdr[p, qh * 2 + b]
        nc.sync.dma_start(rows[None, :], zrc[:])
        for h in heads:
            pb = 64 * (h % 2)
            zb = zs.tile([128, 512], F32, tag="zb", name="zb")
            nc.sync.dma_start(
                zb[:],
                rows[(h % 2) * 512:(h % 2 + 1) * 512][None, :]
                .broadcast_to([128, 512]))
            nc.vector.tensor_tensor(o_all[pb:pb + 64, p, cs],
                                    o_all[pb:pb + 64, p, cs],
                                    zb[pb:pb + 64, :], OP.mult)

    def emit_av(j, e_sbs):
        if j == 0:
            # allocated here (not at S-emission) so the PE's S stream never
            # queues behind the previous pair's accumulator release
            state["o"] = {h: ps.tile([65, QH], F32, tag="o", name=f"o{h}_{qh}")
                          for h in heads}
        o_ps = state["o"]
        coff = max(0, 128 * j - QH * qh)
        c0 = coff
        for h in heads:
            c0 = coff
            while c0 < QH:
                hi = min(QH, (c0 // 512 + 1) * 512)
                nc.tensor.matmul(
                    o_ps[h][:, c0:hi],
                    lhsT=v_sb[:, j, h, :],
                    rhs=e_sbs[h][:, c0 - coff:hi - coff],
                    start=(j == 0), stop=(j == 8 * qh + 3 + 4 * (c0 // 512)),
                    skip_group_check=True)
                c0 = hi
        if j == 8 * qh + 3:
            evict_half(o_ps, 0)
        elif j == jmax - 1:
            evict_half(state.pop("o"), 1)

    def j_unit(j):
        # software-pipelined: emit S/exp for j, then the AV for j-1, so
        # every AV's exp finished a full period earlier and the PE never
        # waits on the scalar engine in steady state
        qstart = max(QH * qh, 128 * j)
        n = QH * (qh + 1) - qstart
        e_sbs = {}
        for h in heads:
            s_ps = ps.tile([128, QH], F32, tag="s", name=f"s{h}")
            for c in range(0, n, 512):
                cn = min(512, n - c)
                nc.tensor.matmul(
                    s_ps[:, c:c + cn],
                    lhsT=kT_sb[:, h, j * 128:(j + 1) * 128],
                    rhs=qT_sb[:, p, qstart + c:qstart + c + cn],
                    start=True, stop=True)
            e_sb = es.tile([128, QH], BF16, tag="e", name=f"e{h}")
            nc.scalar.activation(e_sb[:, 0:n], s_ps[:, 0:n], AF.Exp,
                                 scale=SCALE)
            if j >= 8 * qh:  # diagonal block: zero out key > query
                nc.vector.tensor_tensor(
                    e_sb[:, 0:128], e_sb[:, 0:128], mask_sb[:], OP.mult)
            e_sbs[h] = e_sb
        pend = state.get("pend")
        state["pend"] = (j, e_sbs)
        if pend is not None:
            emit_av(*pend)

    def flush_unit():
        emit_av(*state.pop("pend"))

    return [lambda j=j: j_unit(j) for j in range(jmax)] + [flush_unit]


def build_nc():
    nc = bacc.Bacc("TRN2", target_bir_lowering=False, debug=False,
                   enable_asserts=False, num_devices=N_CORES)
    xT = nc.dram_tensor("xT", [D, T], BF16, kind="ExternalInput").ap()
    wqT = nc.dram_tensor("wqT", [D, 512], BF16, kind="ExternalInput").ap()
    wkT = nc.dram_tensor("wkT", [D, 512], BF16, kind="ExternalInput").ap()
    wvT = nc.dram_tensor("wvT", [D, 512], BF16, kind="ExternalInput").ap()
    wpT = nc.dram_tensor("wpT", [512, D], BF16, kind="ExternalInput").ap()
    mask = nc.dram_tensor("mask", [128, 128], BF16, kind="ExternalInput").ap()
    zdr = nc.dram_tensor("zdr", [4, 4, 2 * 512], F32).ap()
    y = nc.dram_tensor("y", [T, D], BF16, kind="ExternalOutput").ap()

    from contextlib import ExitStack
    with tile.TileContext(nc) as tc, ExitStack() as ctx:
        per = ctx.enter_context(tc.tile_pool(name="per", bufs=1))

        qT_sb = per.tile([128, 4, T], BF16, tag="qT")
        kT_sb = per.tile([128, HG, T], BF16, tag="kT")   # zero-padded K=128
        v_sb = per.tile([128, 16, HG, 65], BF16, tag="v")
        mask_sb = per.tile([128, 128], BF16, tag="mask")
        o_all = per.tile([128, 4, T], BF16, tag="oall")
        wp_sb = per.tile([128, 4, D], BF16, tag="wp")

        # zero the unused contraction half of each padded k strip; ones
        # column in V for the softmax denominators
        for h in range(HG):
            dead = slice(64, 128) if h % 2 == 0 else slice(0, 64)
            nc.vector.memset(kT_sb[dead, h, :], 0)
        nc.vector.memset(v_sb[:], 1.0)
        nc.sync.dma_start(mask_sb[:], mask[:])

        # weight + x tile loads (consumption order: wk, x half0, wq, wv, ...)
        wpl = ctx.enter_context(tc.tile_pool(name="wpl", bufs=1))
        wk_sb = wpl.tile([128, 8, 512], BF16, tag="wk")
        wq_sb = wpl.tile([128, 8, 512], BF16, tag="wq")
        wv_sb = wpl.tile([128, 8, 512], BF16, tag="wv")
        xT_r = xT.rearrange("(ko ki) t -> ki ko t", ki=128)
        wk_r = wkT.rearrange("(ko ki) n -> ki ko n", ki=128)
        # x as four [128, 8, 512] blocks, weights in 1-2 big DMAs: the sync
        # engine dispatches each DMA instruction serially (~0.6us), so few
        # large transfers beat many small ones at the head of the kernel
        xp = ctx.enter_context(tc.tile_pool(name="xp", bufs=1))
        xh = [[None, None], [None, None]]
        for hf in range(2):
            for c2 in range(2):
                xh[hf][c2] = xp.tile([128, 8, 512], BF16, tag=f"x{hf}{c2}",
                                     name=f"x{hf}{c2}")
        nc.sync.dma_start(wk_sb[:, 0:4, :], wk_r[:, 0:4])
        nc.sync.dma_start(xh[0][0][:], xT_r[:, :, 0:512])
        nc.sync.dma_start(wk_sb[:, 4:8, :], wk_r[:, 4:8])
        nc.sync.dma_start(xh[0][1][:], xT_r[:, :, 512:QH])
        nc.sync.dma_start(wq_sb[:],
                          wqT.rearrange("(ko ki) n -> ki ko n", ki=128))
        nc.sync.dma_start(xh[1][0][:], xT_r[:, :, QH:QH + 512])
        nc.sync.dma_start(xh[1][1][:], xT_r[:, :, QH + 512:T])
        nc.sync.dma_start(wv_sb[:],
                          wvT.rearrange("(ko ki) n -> ki ko n", ki=128))
        nc.sync.dma_start(wp_sb[:],
                          wpT.rearrange("(ko ki) n -> ki ko n", ki=128))

        # PSUM: tag "s" 2 banks x 2 bufs + tag "o" 2 banks x 2 bufs = 8 banks
        ps = ctx.enter_context(tc.tile_pool(name="ps", bufs=2, space="PSUM"))
        es = ctx.enter_context(tc.tile_pool(name="es", bufs=6))
        zs = ctx.enter_context(tc.tile_pool(name="zs", bufs=3))
        yo = ctx.enter_context(tc.tile_pool(name="yo", bufs=2))

        def proj_unit(m):
            ms = slice(m * 128, (m + 1) * 128)
            # the last tiles alternate onto the freed attention accumulator
            # slots so psum rotation never gates the tail
            tag = "o" if m >= 12 and m % 2 == 0 else "s"
            yp = ps.tile([128, QH], F32, tag=tag, name="yp")
            for ch in range(2):
                sl = slice(ch * 512, (ch + 1) * 512)
                for kk in range(4):
                    nc.tensor.matmul(
                        yp[:, sl], lhsT=o_all[:, kk, ms],
                        rhs=wp_sb[:, kk, sl], start=(kk == 0), stop=(kk == 3))
            y_sb = yo.tile([128, D], BF16, tag="y", name="ysb")
            nc.vector.tensor_copy(y_sb[:], yp[:])
            nc.sync.dma_start(y[ms, :], y_sb[:])

        # phase 1: k/q/v of token half 0 (evictions on the idle scalar engine)
        for u in _kqv_units(nc, 0, xh[0], wk_sb, wq_sb, wv_sb,
                            qT_sb, kT_sb, v_sb, ps, nc.scalar):
            u()
        # phase 2: query-half-0 attention, second-half k/q/v interleaved as
        # PE filler (its evictions on the vector engine; ACT is doing exps)
        attn0 = []
        for p in range(4):
            attn0 += _attn_units(nc, p, 0, qT_sb, kT_sb, v_sb, mask_sb,
                                 o_all, ps, es, zs, zdr, {})
        kqv1 = _kqv_units(nc, 1, xh[1], wk_sb, wq_sb, wv_sb,
                          qT_sb, kT_sb, v_sb, ps, nc.vector)
        _interleave(attn0, kqv1)
        # phase 3: query-half-1 attention with the first 8 projection tiles
        # (tokens 0..1023, which depend only on query-half-0 output) as filler
        ones_sb = per.tile([1, 128], F32, tag="ones")
        nc.vector.memset(ones_sb[:], 1.0)
        attn1 = []
        for p in range(4):
            attn1 += _attn_units(nc, p, 1, qT_sb, kT_sb, v_sb, mask_sb,
                                 o_all, ps, es, zs, zdr, {},
                                 ones_sb=ones_sb if p == 3 else None)
        # tokens 0..1023 depend only on query-half-0 output: usable as filler
        # anywhere in attn1. Tokens 1024..1535 become ready after the last
        # pair's bank-0 eviction (unit index 48 + 11); emit those right after.
        # v of token-half 1 is first needed at pair 0's j=8: interleave its
        # units into pair 0's j<8 steps, the early projection tiles after.
        proj07 = [lambda m=m: proj_unit(m) for m in range(8)]
        _interleave(attn1[:64], proj07)
        for i, u in enumerate(attn1[64:]):
            u()
            proj_unit(8 + i)
        # phase 4: remaining projection tiles
        for m in range(12, 16):
            proj_unit(m)

    nc.compile()
    return nc


# ---------------------------------------------------------------------------
# host-side sharding + entry point
# ---------------------------------------------------------------------------

_NC_CACHE = {}


def _get_nc():
    if "nc" not in _NC_CACHE:
        _NC_CACHE["nc"] = build_nc()
    return _NC_CACHE["nc"]


def _make_in_maps(x, Wq, Wk, Wv, Wp):
    bf = mybir.dt.np(BF16)
    x = np.asarray(x, dtype=np.float32)
    Wq = np.asarray(Wq, dtype=np.float32)
    Wk = np.asarray(Wk, dtype=np.float32)
    Wv = np.asarray(Wv, dtype=np.float32)
    Wp = np.asarray(Wp, dtype=np.float32)

    mask = np.zeros((128, 128), dtype=np.float32)
    k_idx = np.arange(128)[:, None]
    q_idx = np.arange(128)[None, :]
    mask[q_idx >= k_idx] = 1.0
    mask = mask.astype(bf)

    xTs = [np.ascontiguousarray(x[b].T).astype(bf) for b in range(B)]
    in_maps = []
    for c in range(N_CORES):
        b, g = c // 2, c % 2
        rows = slice(512 * g, 512 * (g + 1))
        m = {
            "xT": xTs[b],
            "wqT": np.ascontiguousarray(Wq[rows, :].T).astype(bf),
            "wkT": np.ascontiguousarray(Wk[rows, :].T).astype(bf),
            "wvT": np.ascontiguousarray(Wv[rows, :].T).astype(bf),
            "wpT": np.ascontiguousarray(Wp[:, rows].T).astype(bf),
            "mask": mask,
        }
        in_maps.append(m)
    return in_maps


def kernel(x, Wq, Wk, Wv, Wp, bp, _trace=False):
    nc = _get_nc()
    in_maps = _make_in_maps(x, Wq, Wk, Wv, Wp)
    res = _run_spmd(nc, in_maps, trace=_trace)
    bp = np.asarray(bp, dtype=np.float32)
    out = np.empty((B, T, D), dtype=np.float32)
    for b in range(B):
        out[b] = (res.results[2 * b]["y"].astype(np.float32)
                  + res.results[2 * b + 1]["y"].astype(np.float32) + bp)
    if _trace:
        kernel.last_results = res
    return out


# revision 56
# speedup vs baseline: 1.2230x; 1.2230x over previous
"""Trainium2 Bass kernel for causal multi-head attention + output projection.

Problem (hardcoded): x[4, 2048, 1024] fp32, 16 heads, head_dim 64, causal,
torch-Linear convention (y = x @ W.T), output projection with bias.

Sharding over 8 NeuronCores: batch (4) x head-group (2 groups of 8 heads).
Each core computes q/k/v for its 8 heads of its batch, causal attention in
the S^T layout (keys on partitions, queries on free dim; softmax denominators
produced by an appended ones-column in V), then a partial output projection
over its 512 head-dims for all 2048 queries. The host sums the two partial
projections per batch and adds the bias (tensor-parallel unshard).

All matmul operands are bf16 (fp32 PSUM accumulation). S matmuls zero-pad the
64-dim head contraction to K=128 so every matmul runs the full 128x128 array
configuration (no PE tile-mode switches, which drain the array). Emission
round-robins independent GEMM work into the attention windows (k/q/v of the
second token half into query-half-0 attention, the first 8 projection tiles
into query-half-1 attention) so the in-order PE stream never starves while
the scalar engine works through the exp strips.
"""
import os
import sys
import types

import numpy as np

import concourse.bass as bass
import concourse.mybir as mybir
import concourse.tile as tile
from concourse import bacc, bass_utils

BF16 = mybir.dt.bfloat16
F32 = mybir.dt.float32
AF = mybir.ActivationFunctionType
OP = mybir.AluOpType

B, T, D = 4, 2048, 1024
H, HD = 16, 64
HG = 8          # heads per core
QH = T // 2     # query half
N_CORES = 8
SCALE = 1.0 / 8.0
SKIP_NORM = False


# ---------------------------------------------------------------------------
# environment glue
# ---------------------------------------------------------------------------

def _install_ntff_hook():
    if 'antenv.axon_hooks' in sys.modules:
        return
    try:
        from trn_agent_boot.trn_boot import _ntff_profile_via_ctypes
        hook = _ntff_profile_via_ctypes('/opt/axon/libaxon_pjrt.so')
    except Exception:
        hook = None
    mod = types.ModuleType('antenv.axon_hooks')
    mod.get_axon_ntff_profile_hook = lambda: hook
    mod.set_axon_ntff_profile_hook = lambda h: None
    sys.modules['antenv.axon_hooks'] = mod


def _run_spmd(nc, in_maps, trace=False):
    from concourse.bass_interp import get_hw_module
    bass_utils.upload_artifacts = lambda tmpdir: tmpdir
    if trace:
        _install_ntff_hook()
    old_m = nc.m
    nc.m = get_hw_module(nc.m)
    try:
        return bass_utils.run_bass_kernel_spmd(
            nc, in_maps, core_ids=list(range(N_CORES)),
            trace=trace, trace_cores=[0] if trace else None,
        )
    finally:
        nc.m = old_m


def _interleave(main_units, filler_units):
    """Emit main_units in order, spreading filler_units evenly between them."""
    n_main, n_fill = len(main_units), len(filler_units)
    fi = 0
    for i, u in enumerate(main_units):
        u()
        want = ((i + 1) * n_fill) // max(1, n_main)
        while fi < want:
            filler_units[fi]()
            fi += 1
    while fi < n_fill:
        filler_units[fi]()
        fi += 1


# ---------------------------------------------------------------------------
# kernel program
# ---------------------------------------------------------------------------

def _kqv_units(nc, hf, xh, wk_sb, wq_sb, wv_sb, qT_sb, kT_sb, v_sb,
               ps, evict_eng):
    """Unit closures for k, q projections of token-half hf and v of its 8
    token blocks. Each unit is ~4096 PE cycles into one psum bank."""
    cbase = hf * QH
    units = []
    if evict_eng is nc.scalar:
        evict = nc.scalar.copy
    else:
        evict = nc.vector.tensor_copy

    def kq_unit(w_sb, dst, padded, m, ch):
        pt = ps.tile([128, QH], F32, tag="s", name="pt")
        half = pt[:, 0:512]
        for kk in range(8):
            nc.tensor.matmul(
                half[:], lhsT=w_sb[:, kk, m * 128:(m + 1) * 128],
                rhs=xh[ch][:, kk, :], start=(kk == 0), stop=(kk == 7))
        cs = slice(cbase + ch * 512, cbase + (ch + 1) * 512)
        if padded:
            evict(dst[0:64, 2 * m, cs], half[0:64, :])
            evict(dst[64:128, 2 * m + 1, cs], half[64:128, :])
        else:
            evict(dst[:, m, cs], half[:])

    def v_unit(mm):
        m = hf * 8 + mm
        pt = ps.tile([128, QH], F32, tag="s", name="pt")
        half = pt[:, 0:512]
        for kk in range(8):
            nc.tensor.matmul(
                half[:],
                lhsT=xh[mm // 4][:, kk, (mm % 4) * 128:(mm % 4 + 1) * 128],
                rhs=wv_sb[:, kk, :], start=(kk == 0), stop=(kk == 7))
        evict(v_sb[:, m, :, 0:64],
              half[:].rearrange("p (h d) -> p h d", h=HG))

    for w_sb, dst, padded in ((wk_sb, kT_sb, True), (wq_sb, qT_sb, False)):
        for m in range(4):
            for ch in range(2):
                units.append(lambda w=w_sb, d=dst, p=padded, m=m, c=ch:
                             kq_unit(w, d, p, m, c))
    for mm in range(8):
        units.append(lambda mm=mm: v_unit(mm))
    return units


def _attn_units(nc, p, qh, qT_sb, kT_sb, v_sb, mask_sb, o_all,
                ps, es, zs, zdr, state, ones_sb=None):
    """Unit closures for heads (2p, 2p+1), query half qh: one unit per
    j-step (S + exp + mask + AV for both heads) plus an eviction unit."""
    heads = (2 * p, 2 * p + 1)
    jmax = 8 * qh + 8
    fast = ones_sb is not None  # last pair: broadcast 1/Z on the idle PE

    def evict_half(o_ps, b):
        # Columns [512b, 512b+512) of the o accumulators got their last AV
        # write at j = 8*qh + 3 + 4*b: evict un-normalized (frees the PSUM
        # bank early), then normalize in SBUF once the reciprocal
        # denominators come back from the DRAM broadcast bounce.
        cs = slice(QH * qh + 512 * b, QH * qh + 512 * (b + 1))
        ps_cs = slice(512 * b, 512 * (b + 1))
        zst = zs.tile([1, 2 * 512], F32, tag="zst", name="zst")
        # bank-1 eviction lands at the pair boundary where the scalar engine
        # sits between exp streams: run its copies there so ACT and DVE
        # drain the accumulators in parallel
        ocopy = nc.scalar.copy if b == 1 else nc.vector.tensor_copy
        for h in heads:
            pb = 64 * (h % 2)
            ocopy(o_all[pb:pb + 64, p, cs], o_ps[h][0:64, ps_cs])
            if not SKIP_NORM:
                nc.vector.tensor_copy(zst[:, (h % 2) * 512:(h % 2 + 1) * 512],
                                      o_ps[h][64:65, ps_cs])
        if SKIP_NORM:
            return
        zrc = zs.tile([1, 2 * 512], F32, tag="zrc", name="zrc")
        nc.vector.reciprocal_approx_fast(zrc[:], zst[:])
        if fast:
            zb_ps = ps.tile([128, QH], F32, tag="s", name="zbps")
            for c in (0, 512):
                nc.tensor.matmul(zb_ps[:, c:c + 512], lhsT=ones_sb[:],
                                 rhs=zrc[:, c:c + 512], start=True, stop=True)
            for h in heads:
                pb = 64 * (h % 2)
                zsl = slice((h % 2) * 512, (h % 2 + 1) * 512)
                nc.vector.tensor_tensor(o_all[pb:pb + 64, p, cs],
                                        o_all[pb:pb + 64, p, cs],
                                        zb_ps[pb:pb + 64, zsl], OP.mult)
            return
        rows = zdr[p, qh * 2 + b]
        nc.sync.dma_start(rows[None, :], zrc[:])
        for h in heads:
            pb = 64 * (h % 2)
            zb = zs.tile([128, 512], F32, tag="zb", name="zb")
            nc.sync.dma_start(
                zb[:],
                rows[(h % 2) * 512:(h % 2 + 1) * 512][None, :]
                .broadcast_to([128, 512]))
            nc.vector.tensor_tensor(o_all[pb:pb + 64, p, cs],
                                    o_all[pb:pb + 64, p, cs],
                                    zb[pb:pb + 64, :], OP.mult)

    def emit_av(j, e_sbs):
        if j == 0:
            # allocated here (not at S-emission) so the PE's S stream never
            # queues behind the previous pair's accumulator release
            state["o"] = {h: ps.tile([65, QH], F32, tag="o", name=f"o{h}_{qh}")
                          for h in heads}
        o_ps = state["o"]
        coff = max(0, 128 * j - QH * qh)
        c0 = coff
        for h in heads:
            c0 = coff
            while c0 < QH:
                hi = min(QH, (c0 // 512 + 1) * 512)
                nc.tensor.matmul(
                    o_ps[h][:, c0:hi],
                    lhsT=v_sb[:, j, h, :],
                    rhs=e_sbs[h][:, c0 - coff:hi - coff],
                    start=(j == 0), stop=(j == 8 * qh + 3 + 4 * (c0 // 512)),
                    skip_group_check=True)
                c0 = hi


    def j_unit(j):
        # software-pipelined: emit S/exp for j, then the AV for j-1, so
        # every AV's exp finished a full period earlier and the PE never
        # waits on the scalar engine in steady state
        qstart = max(QH * qh, 128 * j)
        n = QH * (qh + 1) - qstart
        e_sbs = {}
        for h in heads:
            s_ps = ps.tile([128, QH], F32, tag="s", name=f"s{h}")
            for c in range(0, n, 512):
                cn = min(512, n - c)
                nc.tensor.matmul(
                    s_ps[:, c:c + cn],
                    lhsT=kT_sb[:, h, j * 128:(j + 1) * 128],
                    rhs=qT_sb[:, p, qstart + c:qstart + c + cn],
                    start=True, stop=True)
            e_sb = es.tile([128, QH], BF16, tag="e", name=f"e{h}")
            nc.scalar.activation(e_sb[:, 0:n], s_ps[:, 0:n], AF.Exp,
                                 scale=SCALE)
            if j >= 8 * qh:  # diagonal block: zero out key > query
                nc.vector.tensor_tensor(
                    e_sb[:, 0:128], e_sb[:, 0:128], mask_sb[:], OP.mult)
            e_sbs[h] = e_sb
        pend = state.get("pend")
        state["pend"] = (j, e_sbs)
        if pend is not None:
            emit_av(*pend)

    def evict0_unit():
        # bank 0 is final after j = 8*qh+3; emitting its eviction at the
        # pair's end keeps the DVE/ACT queues clear of bulk copies while
        # the diagonal masks (on the AV critical path) stream
        evict_half(state["o"], 0)

    def flush_unit():
        emit_av(*state.pop("pend"))

    def evict1_unit():
        # emitted two j-units into the NEXT pair so the copies never delay
        # that pair's first exps; the o slots release in its S-phase shadow
        evict_half(state.pop("o"), 1)

    units = [lambda j=j: j_unit(j) for j in range(jmax)]
    units += [evict0_unit, flush_unit]
    return units, evict1_unit


def build_nc():
    nc = bacc.Bacc("TRN2", target_bir_lowering=False, debug=False,
                   enable_asserts=False, num_devices=N_CORES)
    xT = nc.dram_tensor("xT", [D, T], BF16, kind="ExternalInput").ap()
    wqT = nc.dram_tensor("wqT", [D, 512], BF16, kind="ExternalInput").ap()
    wkT = nc.dram_tensor("wkT", [D, 512], BF16, kind="ExternalInput").ap()
    wvT = nc.dram_tensor("wvT", [D, 512], BF16, kind="ExternalInput").ap()
    wpT = nc.dram_tensor("wpT", [512, D], BF16, kind="ExternalInput").ap()
    mask = nc.dram_tensor("mask", [128, 128], BF16, kind="ExternalInput").ap()
    zdr = nc.dram_tensor("zdr", [4, 4, 2 * 512], F32).ap()
    y = nc.dram_tensor("y", [T, D], BF16, kind="ExternalOutput").ap()

    from contextlib import ExitStack
    with tile.TileContext(nc) as tc, ExitStack() as ctx:
        per = ctx.enter_context(tc.tile_pool(name="per", bufs=1))

        qT_sb = per.tile([128, 4, T], BF16, tag="qT")
        kT_sb = per.tile([128, HG, T], BF16, tag="kT")   # zero-padded K=128
        v_sb = per.tile([128, 16, HG, 65], BF16, tag="v")
        mask_sb = per.tile([128, 128], BF16, tag="mask")
        o_all = per.tile([128, 4, T], BF16, tag="oall")
        wp_sb = per.tile([128, 4, D], BF16, tag="wp")

        # zero the unused contraction half of each padded k strip; ones
        # column in V for the softmax denominators
        for h in range(HG):
            dead = slice(64, 128) if h % 2 == 0 else slice(0, 64)
            nc.vector.memset(kT_sb[dead, h, :], 0)
        nc.vector.memset(v_sb[:], 1.0)
        nc.sync.dma_start(mask_sb[:], mask[:])

        # weight + x tile loads (consumption order: wk, x half0, wq, wv, ...)
        wpl = ctx.enter_context(tc.tile_pool(name="wpl", bufs=1))
        wk_sb = wpl.tile([128, 8, 512], BF16, tag="wk")
        wq_sb = wpl.tile([128, 8, 512], BF16, tag="wq")
        wv_sb = wpl.tile([128, 8, 512], BF16, tag="wv")
        xT_r = xT.rearrange("(ko ki) t -> ki ko t", ki=128)
        wk_r = wkT.rearrange("(ko ki) n -> ki ko n", ki=128)
        # x as four [128, 8, 512] blocks, weights in 1-2 big DMAs: the sync
        # engine dispatches each DMA instruction serially (~0.6us), so few
        # large transfers beat many small ones at the head of the kernel
        xp = ctx.enter_context(tc.tile_pool(name="xp", bufs=1))
        xh = [[None, None], [None, None]]
        for hf in range(2):
            for c2 in range(2):
                xh[hf][c2] = xp.tile([128, 8, 512], BF16, tag=f"x{hf}{c2}",
                                     name=f"x{hf}{c2}")
        nc.sync.dma_start(wk_sb[:, 0:4, :], wk_r[:, 0:4])
        nc.sync.dma_start(xh[0][0][:], xT_r[:, :, 0:512])
        nc.sync.dma_start(wk_sb[:, 4:8, :], wk_r[:, 4:8])
        nc.sync.dma_start(xh[0][1][:], xT_r[:, :, 512:QH])
        nc.sync.dma_start(wq_sb[:],
                          wqT.rearrange("(ko ki) n -> ki ko n", ki=128))
        nc.sync.dma_start(xh[1][0][:], xT_r[:, :, QH:QH + 512])
        nc.sync.dma_start(xh[1][1][:], xT_r[:, :, QH + 512:T])
        nc.sync.dma_start(wv_sb[:],
                          wvT.rearrange("(ko ki) n -> ki ko n", ki=128))
        nc.sync.dma_start(wp_sb[:],
                          wpT.rearrange("(ko ki) n -> ki ko n", ki=128))

        # PSUM: tag "s" 2 banks x 2 bufs + tag "o" 2 banks x 2 bufs = 8 banks
        ps = ctx.enter_context(tc.tile_pool(name="ps", bufs=2, space="PSUM"))
        es = ctx.enter_context(tc.tile_pool(name="es", bufs=4))
        zs = ctx.enter_context(tc.tile_pool(name="zs", bufs=3))
        yo = ctx.enter_context(tc.tile_pool(name="yo", bufs=2))

        def proj_unit(m):
            ms = slice(m * 128, (m + 1) * 128)
            # the last tiles alternate onto the freed attention accumulator
            # slots so psum rotation never gates the tail
            tag = "o" if m >= 12 and m % 2 == 0 else "s"
            yp = ps.tile([128, QH], F32, tag=tag, name="yp")
            for ch in range(2):
                sl = slice(ch * 512, (ch + 1) * 512)
                for kk in range(4):
                    nc.tensor.matmul(
                        yp[:, sl], lhsT=o_all[:, kk, ms],
                        rhs=wp_sb[:, kk, sl], start=(kk == 0), stop=(kk == 3))
            y_sb = yo.tile([128, D], BF16, tag="y", name="ysb")
            nc.vector.tensor_copy(y_sb[:], yp[:])
            nc.sync.dma_start(y[ms, :], y_sb[:])

        # phase 1: k/q/v of token half 0 (evictions on the idle scalar engine)
        for u in _kqv_units(nc, 0, xh[0], wk_sb, wq_sb, wv_sb,
                            qT_sb, kT_sb, v_sb, ps, nc.scalar):
            u()
        # phase 2: query-half-0 attention, second-half k/q/v interleaved as
        # PE filler (its evictions on the vector engine; ACT is doing exps).
        # Each pair's half-1 eviction is deferred past the next pair's first
        # unit so its copies never stall that pair's exp stream.
        attn0 = []
        pend_ev = None
        for p in range(4):
            units, ev1 = _attn_units(nc, p, 0, qT_sb, kT_sb, v_sb, mask_sb,
                                     o_all, ps, es, zs, zdr, {})
            if pend_ev is not None:
                units = units[:1] + [pend_ev] + units[1:]
            attn0 += units
            pend_ev = ev1
        attn0.append(pend_ev)
        kqv1 = _kqv_units(nc, 1, xh[1], wk_sb, wq_sb, wv_sb,
                          qT_sb, kT_sb, v_sb, ps, nc.vector)
        _interleave(attn0, kqv1)
        # phase 3: query-half-1 attention with the first 8 projection tiles
        # (tokens 0..1023, which depend only on query-half-0 output) as filler
        ones_sb = per.tile([1, 128], F32, tag="ones")
        nc.vector.memset(ones_sb[:], 1.0)
        attn1 = []
        pend_ev = None
        for p in range(4):
            units, ev1 = _attn_units(nc, p, 1, qT_sb, kT_sb, v_sb, mask_sb,
                                     o_all, ps, es, zs, zdr, {},
                                     ones_sb=ones_sb if p == 3 else None)
            if pend_ev is not None:
                units = units[:1] + [pend_ev] + units[1:]
            attn1 += units
            pend_ev = ev1
        attn1.append(pend_ev)
        # tokens 0..1023 depend only on query-half-0 output: usable as filler
        # anywhere in attn1
        proj07 = [lambda m=m: proj_unit(m) for m in range(8)]
        _interleave(attn1, proj07)
        # phase 4: remaining projection tiles (need every pair's query-half-1
        # output; the last pair's normalization uses the fast PE broadcast)
        for m in range(8, 16):
            proj_unit(m)

    nc.compile()
    return nc


# ---------------------------------------------------------------------------
# host-side sharding + entry point
# ---------------------------------------------------------------------------

_NC_CACHE = {}


def _get_nc():
    if "nc" not in _NC_CACHE:
        _NC_CACHE["nc"] = build_nc()
    return _NC_CACHE["nc"]


def _make_in_maps(x, Wq, Wk, Wv, Wp):
    bf = mybir.dt.np(BF16)
    x = np.asarray(x, dtype=np.float32)
    Wq = np.asarray(Wq, dtype=np.float32)
    Wk = np.asarray(Wk, dtype=np.float32)
    Wv = np.asarray(Wv, dtype=np.float32)
    Wp = np.asarray(Wp, dtype=np.float32)

    mask = np.zeros((128, 128), dtype=np.float32)
    k_idx = np.arange(128)[:, None]
    q_idx = np.arange(128)[None, :]
    mask[q_idx >= k_idx] = 1.0
    mask = mask.astype(bf)

    xTs = [np.ascontiguousarray(x[b].T).astype(bf) for b in range(B)]
    in_maps = []
    for c in range(N_CORES):
        b, g = c // 2, c % 2
        rows = slice(512 * g, 512 * (g + 1))
        m = {
            "xT": xTs[b],
            "wqT": np.ascontiguousarray(Wq[rows, :].T).astype(bf),
            "wkT": np.ascontiguousarray(Wk[rows, :].T).astype(bf),
            "wvT": np.ascontiguousarray(Wv[rows, :].T).astype(bf),
            "wpT": np.ascontiguousarray(Wp[:, rows].T).astype(bf),
            "mask": mask,
        }
        in_maps.append(m)
    return in_maps


def kernel(x, Wq, Wk, Wv, Wp, bp, _trace=False):
    nc = _get_nc()
    in_maps = _make_in_maps(x, Wq, Wk, Wv, Wp)
    res = _run_spmd(nc, in_maps, trace=_trace)
    bp = np.asarray(bp, dtype=np.float32)
    out = np.empty((B, T, D), dtype=np.float32)
    for b in range(B):
        out[b] = (res.results[2 * b]["y"].astype(np.float32)
                  + res.results[2 * b + 1]["y"].astype(np.float32) + bp)
    if _trace:
        kernel.last_results = res
    return out
